# revision 4
# baseline (speedup 1.0000x reference)
"""Multi-head causal attention (RoPE + per-head RMSNorm) on 8 TRN2 NeuronCores.

Reference computation (B=4, T=2048, C=1024, H=16, D=64):
    kqv = x @ W_kqv.T ; k,q,v = split(kqv) ; heads ; RoPE(q,k) ; RMSNorm(q,k)
    att = softmax(causal(q k^T / sqrt(D))) ; y = att v ; out = y @ W_proj.T

Sharding: core c -> batch b = c//2, head group g = c%2 (heads 8g..8g+8).
Each core computes a partial out[b] over its 8 heads' channels; host sums the
two partials per batch.

v3 changes vs the 900us baseline (driven by TimelineSim engine occupancy:
ACT engine was 81% busy, 135us of it activation-table reloads from
Exp<->Sqrt switching; PE/ACT phases serialized):
  - the ONLY ACT functions used are Exp and Ln, which share one activation
    table (natural_log_exp_and_others). A Bacc subclass pins the
    table-choice pass to that set: exactly one table load for the whole
    kernel. rsqrt for RMSNorm is exp(-0.5*ln(ss)); the 8 = sqrt(D) factors
    fold into the broadcast weight matrices.
  - RMS stats for all 4 head-pairs of a chunk are computed into shared PSUM
    tiles (rows 32p..32p+2) and ln/exp'd in 2 instructions per q/k instead
    of per-pair partition-sparse ops (ACT cost is free-size bound).
  - softmax denominator: y+denom staged PSUM->SBUF by the (idle) Pool
    engine, fp32 reciprocal on DVE in place, denominator broadcast as an
    f32r matmul (full PE rate at N=512, no fp16-range sqrt trick) written
    back into the just-freed region of the same PSUM bank, one fused
    multiply to fp16 yT.
  - software pipelining: attention (phase B) for chunk t is ACT-bound while
    the projections (phase A) are PE-bound, so phase A of chunk t+1 is
    emitted interleaved between phase-B heads of chunk t. PSUM rings are
    disjoint (psA: kqv/rot/v/outproj, psS: stats+broadcasts, psB: scores,
    psY: y accum + in-place denominator) so the phases only share engines.
  - per-head s-blocks run causal-interior first, diagonal (masked) blocks
    last, so the exp->mask->AV dependency tail overlaps the next head.
"""

import sys

import numpy as np

sys.path.insert(0, "/opt/trn_rl_repo")

B, T, C, H, D = 4, 2048, 1024, 16, 64
N_CORES = 8
HPC = H // 2  # heads per core: 8
TC = 512  # t-chunk (matmul free dim)
NTC = T // TC  # 4
NST = T // 128  # 16 s/t subtiles

_STATE: dict = {}


def _make_bacc():
    import concourse.mybir as mybir
    from concourse import bacc
    from concourse.hw_specs import get_activation_tables
    import bass_rust as _bass_rust

    AF = mybir.ActivationFunctionType
    SHARED = "natural_log_exp_and_others"

    class PinnedTableBacc(bacc.Bacc):
        """Bacc whose activation-table pass serves Exp/Ln only from the one
        set containing both, so the kernel needs a single table load
        (the default pass greedily picks exp_and_others / natural_log and
        thrashes 1.3us reloads on every Exp<->Ln transition)."""

        def insert_act_table_loads(self):
            has_activation = any(
                isinstance(i, mybir.InstActivation)
                for b in self.main_func.blocks
                for i in b.instructions
            )
            if not has_activation:
                return
            tables = []
            for name, funcs in get_activation_tables(self.m.arch).items():
                if name != SHARED:
                    funcs = funcs - {AF.Exp, AF.Ln}
                tables.append((name, funcs))
            _bass_rust.insert_act_table_loads(self, tables)

    return PinnedTableBacc


def _build_nc(loop_n=None):
    import concourse.mybir as mybir
    from concourse.tile import TileContext
    from contextlib import ExitStack

    f16 = mybir.dt.float16
    f32 = mybir.dt.float32
    f32r = mybir.dt.float32r
    AF = mybir.ActivationFunctionType

    nc = _make_bacc()(
        "TRN2",
        target_bir_lowering=False,
        debug=False,
        num_devices=N_CORES,
    )

    xT = nc.dram_tensor("xT", [NTC, 128, 8, TC], f16, kind="ExternalInput")
    wqT = nc.dram_tensor("wqT", [128, 8, 512], f16, kind="ExternalInput")
    wkT = nc.dram_tensor("wkT", [128, 8, 512], f16, kind="ExternalInput")
    wvT = nc.dram_tensor("wvT", [128, 8, 512], f16, kind="ExternalInput")
    wpT = nc.dram_tensor("wpT", [128, 4, 1024], f16, kind="ExternalInput")
    cosd = nc.dram_tensor("cosd", [128, T], f16, kind="ExternalInput")
    sind = nc.dram_tensor("sind", [128, T], f16, kind="ExternalInput")
    maskd = nc.dram_tensor("maskd", [128, 4, TC], f16, kind="ExternalInput")
    p2d = nc.dram_tensor("p2d", [128, 128], f16, kind="ExternalInput")
    ocd = nc.dram_tensor("ocd", [128, 32], f16, kind="ExternalInput")
    obwqd = nc.dram_tensor("obwqd", [98, 128], f16, kind="ExternalInput")
    obwkd = nc.dram_tensor("obwkd", [98, 128], f16, kind="ExternalInput")
    outd = nc.dram_tensor("out", [T, C], f32, kind="ExternalOutput")

    with TileContext(nc) as tc, ExitStack() as ctx:
        const = ctx.enter_context(tc.tile_pool(name="const", bufs=1))
        xpool = ctx.enter_context(tc.tile_pool(name="xp", bufs=2))
        persist = ctx.enter_context(tc.tile_pool(name="persist", bufs=1))
        work = ctx.enter_context(tc.tile_pool(name="work", bufs=3))
        attp = ctx.enter_context(tc.tile_pool(name="attp", bufs=6))
        outp = ctx.enter_context(tc.tile_pool(name="outp", bufs=2))
        psA = ctx.enter_context(tc.tile_pool(name="psA", bufs=2, space="PSUM"))
        psB = ctx.enter_context(tc.tile_pool(name="psB", bufs=2, space="PSUM"))
        psY = ctx.enter_context(tc.tile_pool(name="psY", bufs=2, space="PSUM"))
        psS = ctx.enter_context(tc.tile_pool(name="psS", bufs=2, space="PSUM"))

        # ---- constants ----
        cos_sb = const.tile([128, T], f16, tag="cos")
        nc.sync.dma_start(cos_sb, cosd[:, :])
        sin_sb = const.tile([128, T], f16, tag="sin")
        nc.sync.dma_start(sin_sb, sind[:, :])
        mask_sb = const.tile([128, 4, TC], f16, tag="mask")
        nc.sync.dma_start(mask_sb, maskd[:, :, :])
        p2_sb = const.tile([128, 128], f16, tag="p2")
        nc.sync.dma_start(p2_sb, p2d[:, :])
        oc_sb = const.tile([128, 32], f16, tag="oc")
        nc.sync.dma_start(oc_sb, ocd[:, :])
        obwq_sb = const.tile([98, 128], f16, tag="obwq")
        nc.sync.dma_start(obwq_sb, obwqd[:, :])
        obwk_sb = const.tile([98, 128], f16, tag="obwk")
        nc.sync.dma_start(obwk_sb, obwkd[:, :])
        ones16 = const.tile([65, 64], f16, tag="ones16")
        nc.vector.memset(ones16, 1.0)
        wq_sb = const.tile([128, 8, 512], f16, tag="wq")
        nc.sync.dma_start(wq_sb, wqT[:, :, :])
        wk_sb = const.tile([128, 8, 512], f16, tag="wk")
        nc.sync.dma_start(wk_sb, wkT[:, :, :])
        wv_sb = const.tile([128, 8, 512], f16, tag="wv")
        nc.sync.dma_start(wv_sb, wvT[:, :, :])
        wp_sb = const.tile([128, 4, 1024], f16, tag="wp")
        nc.sync.dma_start(wp_sb, wpT[:, :, :])

        # ---- persistent activations ----
        qT = [
            persist.tile([128, T], f16, tag=f"qT{p}", name=f"qT{p}")
            for p in range(4)
        ]
        kT = [
            persist.tile([128, T], f16, tag=f"kT{p}", name=f"kT{p}")
            for p in range(4)
        ]
        yT = [
            persist.tile([128, T], f16, tag=f"yT{p}", name=f"yT{p}")
            for p in range(4)
        ]
        # v (and the denominator ones-column) carry a 2^-12 scale so that
        # 1/denominator spans [6.6e-4, 4096] -- comfortably fp16 -- letting
        # the denominator broadcast be a plain full-rate fp16 matmul; the
        # 2^12 cancels exactly in y * (1/denom').
        VSC = 2.0 ** -12
        v_sb = persist.tile([128, NST, HPC, 65], f16, tag="v")
        nc.vector.memset(v_sb[:, :, :, 64:65], VSC)

        def kqv_mm(ps, w_sb, p, xt):
            for ci in range(8):
                nc.tensor.matmul(
                    ps,
                    lhsT=w_sb[:, ci, p * 128 : (p + 1) * 128],
                    rhs=xt[:, ci, :],
                    start=(ci == 0),
                    stop=(ci == 7),
                )

        def proj_rope(xt, p, tsl, ss_q, ss_k, ro_q, ro_k):
            """q,k for head pair p: projection, squares into shared stat
            tiles, RoPE combine into staging (normalization applied later)."""
            ps_q = psA.tile([128, TC], f32, tag="kqv")
            kqv_mm(ps_q, wq_sb, p, xt)
            qraw = work.tile([128, TC], f16, tag="qraw")
            nc.vector.tensor_copy(qraw, ps_q)
            sq_q = work.tile([128, TC], f16, tag="sq_q")
            nc.vector.tensor_mul(sq_q, qraw, qraw)
            # RoPE preserves row norms -> sums of squares from pre-RoPE values
            # (eps=1e-6 on rms~1 is far below fp16 noise; dropped).
            # M=32 with zero weight columns 2:32: rows 32p+2..32p+32 get
            # computed zeros, so all 128 stat rows are written (the batched
            # Ln below reads whole tiles; ln(0) rows are never used).
            nc.tensor.matmul(
                ss_q[32 * p : 32 * p + 32, :],
                lhsT=oc_sb,
                rhs=sq_q,
                start=True,
                stop=True,
                tile_position=(0, 32 * p),
            )

            ps_k = psA.tile([128, TC], f32, tag="kqv")
            kqv_mm(ps_k, wk_sb, p, xt)
            kraw = work.tile([128, TC], f16, tag="kraw")
            nc.vector.tensor_copy(kraw, ps_k)
            sq_k = work.tile([128, TC], f16, tag="sq_k")
            nc.vector.tensor_mul(sq_k, kraw, kraw)
            nc.tensor.matmul(
                ss_k[32 * p : 32 * p + 32, :],
                lhsT=oc_sb,
                rhs=sq_k,
                start=True,
                stop=True,
                tile_position=(0, 32 * p),
            )

            # rotate_half via signed permutation matmul on the PE;
            # ro = raw*cos + rot(raw)*sin staged un-normalized (the SBUF-only
            # cos-mul and add run on the otherwise idle Pool engine)
            for raw, dst in ((qraw, ro_q), (kraw, ro_k)):
                rot = psA.tile([128, TC], f32, tag="kqv")
                nc.tensor.matmul(rot, lhsT=p2_sb, rhs=raw, start=True, stop=True)
                qsh = work.tile([128, TC], f16, tag="qsh")
                nc.vector.tensor_mul(qsh, rot, sin_sb[:, tsl])
                t1 = work.tile([128, TC], f16, tag="t1")
                nc.vector.tensor_mul(t1, raw, cos_sb[:, tsl])
                nc.vector.tensor_add(dst, t1, qsh)

        def phase_a_units(tci):
            """Phase A for chunk tci as 8 units interleavable between the
            previous chunk's phase-B heads."""
            tsl = slice(tci * TC, (tci + 1) * TC)
            st8 = {}

            def u_start():
                st8["xt"] = xpool.tile([128, 8, TC], f16, tag="x", name="xt")
                nc.sync.dma_start(st8["xt"], xT[tci])
                st8["ss_q"] = psS.tile([128, TC], f32, tag="s", name="ss_q")
                st8["ss_k"] = psS.tile([128, TC], f32, tag="s", name="ss_k")
                st8["ro_q"] = [
                    work.tile([128, TC], f16, tag=f"roq{p}", name=f"roq{p}")
                    for p in range(4)
                ]
                st8["ro_k"] = [
                    work.tile([128, TC], f16, tag=f"rok{p}", name=f"rok{p}")
                    for p in range(4)
                ]
                proj_rope(
                    st8["xt"], 0, tsl, st8["ss_q"], st8["ss_k"],
                    st8["ro_q"][0], st8["ro_k"][0],
                )

            def u_pair(p):
                def f():
                    proj_rope(
                        st8["xt"], p, tsl, st8["ss_q"], st8["ss_k"],
                        st8["ro_q"][p], st8["ro_k"][p],
                    )
                return f

            def u_v():
                for st in range(4):
                    pv = psA.tile([128, TC], f32, tag="kqv")
                    for ci in range(8):
                        nc.tensor.matmul(
                            pv,
                            lhsT=st8["xt"][:, ci, st * 128 : (st + 1) * 128],
                            rhs=wv_sb[:, ci, :],
                            start=(ci == 0),
                            stop=(ci == 7),
                        )
                    nc.vector.tensor_scalar_mul(
                        v_sb[:, tci * 4 + st, :, 0:64],
                        pv.rearrange("p (h d) -> p h d", h=HPC),
                        VSC,
                    )

            def u_stats():
                # batched rsqrt of rms stats: rr = exp(-0.5 ln ss); the 8
                # from 1/rms = 8/sqrt(ss) is folded into obw. Rows between
                # the 32p..32p+2 stat rows are uninitialized PSUM; their
                # ln/exp results are garbage but never read.
                ln_q = work.tile([128, TC], f32, tag="lnt")
                nc.scalar.activation(ln_q, st8["ss_q"], AF.Ln)
                rr_q = work.tile([128, TC], f16, tag="rrq", bufs=2)
                nc.scalar.activation(rr_q, ln_q, AF.Exp, scale=-0.5)
                ln_k = work.tile([128, TC], f32, tag="lnt")
                nc.scalar.activation(ln_k, st8["ss_k"], AF.Ln)
                rr_k = work.tile([128, TC], f16, tag="rrk", bufs=2)
                nc.scalar.activation(rr_k, ln_k, AF.Exp, scale=-0.5)
                st8["rr_q"], st8["rr_k"] = rr_q, rr_k

            def u_norm(plo, phi):
                def f():
                    # qT/kT = ro * broadcast(8*w*rr)
                    for p in range(plo, phi):
                        bc_q = psS.tile([128, TC], f32, tag="s", name="bc_q")
                        nc.tensor.matmul(
                            bc_q,
                            lhsT=obwq_sb[32 * p : 32 * p + 2, :],
                            rhs=st8["rr_q"][32 * p : 32 * p + 2, :],
                            start=True,
                            stop=True,
                            tile_position=(32 * p, 0),
                        )
                        nc.vector.tensor_mul(qT[p][:, tsl], st8["ro_q"][p], bc_q)
                        bc_k = psS.tile([128, TC], f32, tag="s", name="bc_k")
                        nc.tensor.matmul(
                            bc_k,
                            lhsT=obwk_sb[32 * p : 32 * p + 2, :],
                            rhs=st8["rr_k"][32 * p : 32 * p + 2, :],
                            start=True,
                            stop=True,
                            tile_position=(32 * p, 0),
                        )
                        nc.vector.tensor_mul(kT[p][:, tsl], st8["ro_k"][p], bc_k)
                return f

            return [
                u_start, u_pair(1), u_pair(2), u_pair(3),
                u_v, u_stats, u_norm(0, 2), u_norm(2, 4),
            ]

        def head_b(tci, h):
            """Phase B for one head of chunk tci: scores, exp, causal mask,
            AV accumulation, softmax division."""
            tsl = slice(tci * TC, (tci + 1) * TC)
            n_s = 4 * (tci + 1)
            p, hl = h // 2, h % 2
            hsl = slice(hl * 64, (hl + 1) * 64)
            ps_y = psY.tile([128, TC], f32, tag="y")
            # interior blocks first; diagonal blocks (with their exp->mask->AV
            # dependency tail) last so the tail overlaps the next head
            sis = list(range(4 * tci, n_s)) if tci == 0 else (
                list(range(0, 4 * tci)) + list(range(4 * tci, n_s))
            )
            for idx, si in enumerate(sis):
                # diagonal blocks: columns below the diagonal offset d are
                # fully masked -- compute only the [d, TC) range (the idx==0
                # block is always full width, so every PSUM byte is written)
                delta = si * 128 - tci * TC
                d = max(delta, 0)
                csl = slice(d, TC)
                ps_s = psB.tile([128, TC], f32, tag="sc", name="ps_s")
                nc.tensor.matmul(
                    ps_s[:, csl],
                    lhsT=kT[p][hsl, si * 128 : (si + 1) * 128],
                    rhs=qT[p][hsl, tci * TC + d : (tci + 1) * TC],
                    start=True,
                    stop=True,
                )
                at = attp.tile([128, TC], f16, tag="at")
                nc.scalar.activation(at[:, csl], ps_s[:, csl], AF.Exp, scale=0.125)
                if delta >= 0:
                    # only the leading 128 columns of the valid range cross
                    # the diagonal; the triangular [128,128] mask is the
                    # first block of the offset-0 mask. SBUF-only, so it can
                    # run on the otherwise idle Pool engine.
                    nc.gpsimd.tensor_mul(
                        at[:, d : d + 128],
                        at[:, d : d + 128],
                        mask_sb[:, 0, 0:128],
                    )
                nc.tensor.matmul(
                    ps_y[0:65, csl],
                    lhsT=v_sb[:, si, h, 0:65],
                    rhs=at[:, csl],
                    start=(idx == 0),
                    stop=(idx == n_s - 1),
                )
            # softmax denominator: stage y to SBUF (frees the bank region),
            # fp16 reciprocal straight off PSUM row 64 (the 2^-12 v-scale
            # keeps 1/denom' in fp16 range), broadcast down 64 partitions
            # at full PE rate into the just-staged (hence free) PSUM
            # region, one multiply to fp16 yT.
            ystg = work.tile([65, TC], f32, tag="ystg")
            nc.vector.tensor_copy(ystg[0:64, :], ps_y[0:64, :])
            recw = work.tile([65, TC], f16, tag="recw")
            with nc.allow_low_precision(reason="1/denom' in [6.6e-4,4096]"):
                nc.vector.reciprocal(recw[64:65, :], ps_y[64:65, :])
            nc.tensor.matmul(
                ps_y[0:64, :],
                lhsT=ones16[64:65, :],
                rhs=recw[64:65, :],
                start=True,
                stop=True,
            )
            if hl == 0:
                nc.vector.tensor_mul(
                    yT[p][0:64, tsl], ystg[0:64, :], ps_y[0:64, :]
                )
            else:
                y16 = work.tile([64, TC], f16, tag="y16")
                nc.vector.tensor_mul(y16, ystg[0:64, :], ps_y[0:64, :])
                nc.sync.dma_start(yT[p][64:128, tsl], y16)

        def body():
            for u in phase_a_units(0):
                u()
            for tci in range(NTC):
                nxt = phase_a_units(tci + 1) if tci + 1 < NTC else []
                for h in range(HPC):
                    head_b(tci, h)
                    if h < len(nxt):
                        nxt[h]()

            # ---- phase C: output projection (partials over this core's
            # channels) ----
            for st in range(NST):
                for co in range(2):
                    po = psA.tile([128, TC], f32, tag="kqv")
                    for p in range(4):
                        nc.tensor.matmul(
                            po,
                            lhsT=yT[p][:, st * 128 : (st + 1) * 128],
                            rhs=wp_sb[:, p, co * 512 : (co + 1) * 512],
                            start=(p == 0),
                            stop=(p == 3),
                        )
                    ot = outp.tile([128, TC], f32, tag="o")
                    if co == 0:
                        nc.vector.tensor_copy(ot, po)
                    else:
                        # ACT is idle in phase C; Copy is in every act table
                        nc.scalar.copy(ot, po)
                    nc.sync.dma_start(
                        outd[st * 128 : (st + 1) * 128, co * 512 : (co + 1) * 512],
                        ot,
                    )

        if loop_n is None:
            body()
        else:
            with tc.For_i(0, loop_n, 1):
                body()

    return nc


def _get_nc(loop_n=None):
    key = ("nc", loop_n)
    if key not in _STATE:
        nc = _build_nc(loop_n)
        nc.finalize()
        _STATE[key] = nc
    return _STATE[key]


def _rope_tables():
    inv_freq = 1.0 / (10000.0 ** (np.arange(0, D, 2, dtype=np.float64) / D))
    t_pos = np.arange(T, dtype=np.float64)
    freqs = t_pos[:, None] * inv_freq[None, :]  # [T, 32]
    f2 = np.concatenate([freqs, freqs], axis=-1)  # [T, 64]
    cosT = np.cos(f2).T.astype(np.float16)  # [64, T]
    sinT = np.sin(f2).T.astype(np.float16)
    cos2 = np.concatenate([cosT, cosT], axis=0)  # [128, T]
    sin2 = np.concatenate([sinT, sinT], axis=0)
    return np.ascontiguousarray(cos2), np.ascontiguousarray(sin2)


def _prep_inputs(x, W_kqv, W_proj, q_norm_w, k_norm_w):
    x = np.asarray(x, dtype=np.float32)
    W_kqv = np.asarray(W_kqv, dtype=np.float32)
    W_proj = np.asarray(W_proj, dtype=np.float32)
    q_norm_w = np.asarray(q_norm_w, dtype=np.float32)
    k_norm_w = np.asarray(k_norm_w, dtype=np.float32)

    cos2, sin2 = _rope_tables()

    # causal masks for the 4 diagonal-crossing tile offsets
    si = np.arange(128)[:, None]
    tj = np.arange(TC)[None, :]
    mask = np.stack(
        [(tj >= si + 128 * o).astype(np.float16) for o in range(4)], axis=1
    )  # [128, 4, TC]

    # columns 0/1 sum the two heads' squares; columns 2:32 are zero weights
    # whose computed-zero outputs initialize the unused stat-tile rows
    oc = np.zeros((128, 32), dtype=np.float16)
    oc[0:64, 0] = 1.0
    oc[64:128, 1] = 1.0

    def obw8(w):
        # broadcast weights with the 8 = sqrt(D) of 1/rms folded in, one
        # 2-row block per head pair at partition 32p
        m = np.zeros((98, 128), dtype=np.float16)
        for p in range(4):
            m[32 * p + 0, 0:64] = 8.0 * w
            m[32 * p + 1, 64:128] = 8.0 * w
        return m

    # signed rotate-half permutation (per 64-dim head, stacked twice)
    P = np.zeros((64, 64), dtype=np.float16)
    for i in range(32):
        P[i, i + 32] = -1.0
        P[i + 32, i] = 1.0
    P2 = np.zeros((128, 128), dtype=np.float16)
    P2[0:64, 0:64] = P
    P2[64:128, 64:128] = P
    p2T = np.ascontiguousarray(P2.T)

    def wt_kqv(rows):
        # rows: [512, 1024] -> lhsT layout [128, 8, 512] fp16
        wT = rows.T.astype(np.float16)  # [1024, 512]
        return np.ascontiguousarray(wT.reshape(8, 128, 512).transpose(1, 0, 2))

    Wk, Wq, Wv = W_kqv[0:C], W_kqv[C : 2 * C], W_kqv[2 * C : 3 * C]

    in_maps = []
    for c in range(N_CORES):
        b, g = c // 2, c % 2
        rs = slice(512 * g, 512 * (g + 1))
        xTb = x[b].T.astype(np.float16)  # [C, T]
        xTr = np.ascontiguousarray(
            xTb.reshape(8, 128, NTC, TC).transpose(2, 1, 0, 3)
        )  # [NTC, 128, 8, TC]
        wp = W_proj[:, rs].T.astype(np.float16)  # [512, 1024]
        wpr = np.ascontiguousarray(wp.reshape(4, 128, 1024).transpose(1, 0, 2))
        in_maps.append(
            {
                "xT": xTr,
                "wqT": wt_kqv(Wq[rs]),
                "wkT": wt_kqv(Wk[rs]),
                "wvT": wt_kqv(Wv[rs]),
                "wpT": wpr,
                "cosd": cos2,
                "sind": sin2,
                "maskd": mask,
                "p2d": p2T,
                "ocd": oc,
                "obwqd": obw8(q_norm_w),
                "obwkd": obw8(k_norm_w),
            }
        )
    return in_maps


def _get_runner(loop_n=None):
    """Build (once) a cached jitted SPMD runner mirroring
    bass2jax.run_bass_via_pjrt, so repeated calls reuse the compiled NEFF."""
    key = ("runner", loop_n)
    if key in _STATE:
        return _STATE[key]

    import jax
    import concourse.mybir as mybir
    from concourse import bass2jax
    from concourse.bass2jax import _bass_exec_p, partition_id_tensor
    from jax.experimental.shard_map import shard_map
    from jax.sharding import Mesh, NamedSharding, PartitionSpec

    bass2jax.install_neuronx_cc_hook()
    nc = _get_nc(loop_n)

    partition_name = nc.partition_id_tensor.name if nc.partition_id_tensor else None
    in_names, out_names, out_avals, zero_outs = [], [], [], []
    for alloc in nc.m.functions[0].allocations:
        if not isinstance(alloc, mybir.MemoryLocationSet):
            continue
        name = alloc.memorylocations[0].name
        if alloc.kind == "ExternalInput":
            if name != partition_name:
                in_names.append(name)
        elif alloc.kind == "ExternalOutput":
            shape = tuple(alloc.tensor_shape)
            dtype = mybir.dt.np(alloc.dtype)
            out_names.append(name)
            out_avals.append(jax.core.ShapedArray(shape, dtype))
            zero_outs.append(np.zeros(shape, dtype))
    n_params = len(in_names)
    all_names = in_names + out_names
    if partition_name is not None:
        all_names.append(partition_name)

    def _body(*args):
        operands = list(args)
        if partition_name is not None:
            operands.append(partition_id_tensor())
        outs = _bass_exec_p.bind(
            *operands,
            out_avals=tuple(out_avals),
            in_names=tuple(all_names),
            out_names=tuple(out_names),
            lowering_input_output_aliases=(),
            sim_require_finite=True,
            sim_require_nnan=True,
            nc=nc,
        )
        return tuple(outs)

    devices = jax.devices()[:N_CORES]
    mesh = Mesh(np.asarray(devices), ("core",))
    spec = PartitionSpec("core")
    n_outs = len(out_names)
    sharded = jax.jit(
        shard_map(
            _body,
            mesh=mesh,
            in_specs=(spec,) * (n_params + n_outs),
            out_specs=(spec,) * n_outs,
            check_rep=False,
        ),
        keep_unused=True,
    )
    sharding = NamedSharding(mesh, spec)
    zeros_dev = [
        jax.device_put(
            np.zeros((N_CORES * z.shape[0], *z.shape[1:]), z.dtype), sharding
        )
        for z in zero_outs
    ]
    runner = {
        "sharded": sharded,
        "in_names": in_names,
        "out_names": out_names,
        "out_avals": out_avals,
        "zeros_dev": zeros_dev,
        "sharding": sharding,
    }
    _STATE[key] = runner
    return runner


def _concat_inputs(in_maps, runner):
    return [
        np.concatenate([np.asarray(in_maps[c][n]) for c in range(N_CORES)], axis=0)
        for n in runner["in_names"]
    ]


def _execute(in_maps):
    """Returns list (per core) of {out_name: np.ndarray}."""
    runner = _get_runner()
    concat_in = _concat_inputs(in_maps, runner)
    out_arrs = runner["sharded"](*concat_in, *runner["zeros_dev"])
    return [
        {
            n: np.asarray(out_arrs[i]).reshape(
                N_CORES, *runner["out_avals"][i].shape
            )[c]
            for i, n in enumerate(runner["out_names"])
        }
        for c in range(N_CORES)
    ]


def _wall(runner, in_maps, iters):
    import time
    import jax

    concat_in = [
        jax.device_put(a, runner["sharding"])
        for a in _concat_inputs(in_maps, runner)
    ]
    args = (*concat_in, *runner["zeros_dev"])
    jax.block_until_ready(runner["sharded"](*args))  # warmup
    times = []
    for _ in range(iters):
        t0 = time.perf_counter()
        jax.block_until_ready(runner["sharded"](*args))
        times.append(time.perf_counter() - t0)
    times.sort()
    return times


def _timed(in_maps, iters=20, n_lo=1, n_hi=33):
    """Per-pass HW time via two device-side repeat counts: the dispatch/tunnel
    overhead cancels in the difference."""
    r_lo = _get_runner(None if n_lo == 1 else n_lo)
    r_hi = _get_runner(n_hi)
    t_lo = _wall(r_lo, in_maps, iters)
    t_hi = _wall(r_hi, in_maps, iters)
    k = max(3, iters // 4)
    lo = sum(t_lo[:k]) / k
    hi = sum(t_hi[:k]) / k
    per_pass = (hi - lo) / (n_hi - n_lo)
    return per_pass, lo, hi


def kernel(**inputs):
    in_maps = _prep_inputs(**inputs)
    res = _execute(in_maps)
    out = np.zeros((B, T, C), dtype=np.float32)
    for c in range(N_CORES):
        out[c // 2] += res[c]["out"]
    return out


# revision 6
# speedup vs baseline: 1.0309x; 1.0309x over previous
"""Multi-head causal attention (RoPE + per-head RMSNorm) on 8 TRN2 NeuronCores.

Reference computation (B=4, T=2048, C=1024, H=16, D=64):
    kqv = x @ W_kqv.T ; k,q,v = split(kqv) ; heads ; RoPE(q,k) ; RMSNorm(q,k)
    att = softmax(causal(q k^T / sqrt(D))) ; y = att v ; out = y @ W_proj.T

Sharding: core c -> batch b = c//2, head group g = c%2 (heads 8g..8g+8).
Each core computes a partial out[b] over its 8 heads' channels; host sums the
two partials per batch.

v3 changes vs the 900us baseline (driven by TimelineSim engine occupancy:
ACT engine was 81% busy, 135us of it activation-table reloads from
Exp<->Sqrt switching; PE/ACT phases serialized):
  - the ONLY ACT functions used are Exp and Ln, which share one activation
    table (natural_log_exp_and_others). A Bacc subclass pins the
    table-choice pass to that set: exactly one table load for the whole
    kernel. rsqrt for RMSNorm is exp(-0.5*ln(ss)); the 8 = sqrt(D) factors
    fold into the broadcast weight matrices.
  - RMS stats for all 4 head-pairs of a chunk are computed into shared PSUM
    tiles (rows 32p..32p+2) and ln/exp'd in 2 instructions per q/k instead
    of per-pair partition-sparse ops (ACT cost is free-size bound).
  - softmax denominator: v carries a 2^-12 scale (folded into the v-copy
    and the denominator ones-column; cancels exactly at the division) so
    1/denom' fits fp16; y staged PSUM->SBUF on DVE, fp16 reciprocal off
    PSUM row 64, denominator broadcast as a full-rate fp16 matmul written
    back into the just-freed region of the same PSUM bank, one multiply.
  - software pipelining: attention (phase B) for chunk t is ACT-bound while
    the projections (phase A) are PE-bound, so phase A of chunk t+1 is
    emitted interleaved between phase-B heads of chunk t. PSUM rings are
    disjoint (psA: kqv/rot/v/outproj, psS: stats+broadcasts, psB: scores,
    psY: y accum + in-place denominator) so the phases only share engines.
  - per-head s-blocks run causal-interior first, diagonal (masked) blocks
    last, so the exp->mask->AV dependency tail overlaps the next head.
"""

import sys

import numpy as np

sys.path.insert(0, "/opt/trn_rl_repo")

B, T, C, H, D = 4, 2048, 1024, 16, 64
N_CORES = 8
HPC = H // 2  # heads per core: 8
TC = 512  # t-chunk (matmul free dim)
NTC = T // TC  # 4
NST = T // 128  # 16 s/t subtiles

_STATE: dict = {}


def _make_bacc():
    import concourse.mybir as mybir
    from concourse import bacc
    from concourse.hw_specs import get_activation_tables
    import bass_rust as _bass_rust

    AF = mybir.ActivationFunctionType
    SHARED = "natural_log_exp_and_others"

    class PinnedTableBacc(bacc.Bacc):
        """Bacc whose activation-table pass serves Exp/Ln only from the one
        set containing both, so the kernel needs a single table load
        (the default pass greedily picks exp_and_others / natural_log and
        thrashes 1.3us reloads on every Exp<->Ln transition)."""

        def insert_act_table_loads(self):
            has_activation = any(
                isinstance(i, mybir.InstActivation)
                for b in self.main_func.blocks
                for i in b.instructions
            )
            if not has_activation:
                return
            tables = []
            for name, funcs in get_activation_tables(self.m.arch).items():
                if name != SHARED:
                    funcs = funcs - {AF.Exp, AF.Ln}
                tables.append((name, funcs))
            _bass_rust.insert_act_table_loads(self, tables)

    return PinnedTableBacc


def _build_nc(loop_n=None):
    import concourse.mybir as mybir
    from concourse.tile import TileContext
    from contextlib import ExitStack

    f16 = mybir.dt.float16
    f32 = mybir.dt.float32
    AF = mybir.ActivationFunctionType

    nc = _make_bacc()(
        "TRN2",
        target_bir_lowering=False,
        debug=False,
        num_devices=N_CORES,
    )

    xT = nc.dram_tensor("xT", [NTC, 128, 8, TC], f16, kind="ExternalInput")
    wqT = nc.dram_tensor("wqT", [128, 8, 512], f16, kind="ExternalInput")
    wkT = nc.dram_tensor("wkT", [128, 8, 512], f16, kind="ExternalInput")
    wvT = nc.dram_tensor("wvT", [128, 8, 512], f16, kind="ExternalInput")
    wpT = nc.dram_tensor("wpT", [128, 4, 1024], f16, kind="ExternalInput")
    cosd = nc.dram_tensor("cosd", [128, T], f16, kind="ExternalInput")
    sind = nc.dram_tensor("sind", [128, T], f16, kind="ExternalInput")
    maskd = nc.dram_tensor("maskd", [128, 4, TC], f16, kind="ExternalInput")
    p2d = nc.dram_tensor("p2d", [128, 128], f16, kind="ExternalInput")
    ocd = nc.dram_tensor("ocd", [128, 32], f16, kind="ExternalInput")
    obwqd = nc.dram_tensor("obwqd", [98, 128], f16, kind="ExternalInput")
    obwkd = nc.dram_tensor("obwkd", [98, 128], f16, kind="ExternalInput")
    outd = nc.dram_tensor("out", [T, C], f32, kind="ExternalOutput")

    with TileContext(nc) as tc, ExitStack() as ctx:
        const = ctx.enter_context(tc.tile_pool(name="const", bufs=1))
        xpool = ctx.enter_context(tc.tile_pool(name="xp", bufs=2))
        persist = ctx.enter_context(tc.tile_pool(name="persist", bufs=1))
        work = ctx.enter_context(tc.tile_pool(name="work", bufs=3))
        attp = ctx.enter_context(tc.tile_pool(name="attp", bufs=6))
        outp = ctx.enter_context(tc.tile_pool(name="outp", bufs=2))
        psA = ctx.enter_context(tc.tile_pool(name="psA", bufs=2, space="PSUM"))
        psB = ctx.enter_context(tc.tile_pool(name="psB", bufs=2, space="PSUM"))
        psY = ctx.enter_context(tc.tile_pool(name="psY", bufs=2, space="PSUM"))
        psS = ctx.enter_context(tc.tile_pool(name="psS", bufs=2, space="PSUM"))

        # ---- constants ----
        cos_sb = const.tile([128, T], f16, tag="cos")
        nc.sync.dma_start(cos_sb, cosd[:, :])
        sin_sb = const.tile([128, T], f16, tag="sin")
        nc.sync.dma_start(sin_sb, sind[:, :])
        mask_sb = const.tile([128, 4, TC], f16, tag="mask")
        nc.sync.dma_start(mask_sb, maskd[:, :, :])
        p2_sb = const.tile([128, 128], f16, tag="p2")
        nc.sync.dma_start(p2_sb, p2d[:, :])
        oc_sb = const.tile([128, 32], f16, tag="oc")
        nc.sync.dma_start(oc_sb, ocd[:, :])
        obwq_sb = const.tile([98, 128], f16, tag="obwq")
        nc.sync.dma_start(obwq_sb, obwqd[:, :])
        obwk_sb = const.tile([98, 128], f16, tag="obwk")
        nc.sync.dma_start(obwk_sb, obwkd[:, :])
        ones16 = const.tile([65, 64], f16, tag="ones16")
        nc.vector.memset(ones16, 1.0)
        wq_sb = const.tile([128, 8, 512], f16, tag="wq")
        nc.sync.dma_start(wq_sb, wqT[:, :, :])
        wk_sb = const.tile([128, 8, 512], f16, tag="wk")
        nc.sync.dma_start(wk_sb, wkT[:, :, :])
        wv_sb = const.tile([128, 8, 512], f16, tag="wv")
        nc.sync.dma_start(wv_sb, wvT[:, :, :])
        wp_sb = const.tile([128, 4, 1024], f16, tag="wp")
        nc.sync.dma_start(wp_sb, wpT[:, :, :])

        # ---- persistent activations ----
        qT = [
            persist.tile([128, T], f16, tag=f"qT{p}", name=f"qT{p}")
            for p in range(4)
        ]
        kT = [
            persist.tile([128, T], f16, tag=f"kT{p}", name=f"kT{p}")
            for p in range(4)
        ]
        yT = [
            persist.tile([128, T], f16, tag=f"yT{p}", name=f"yT{p}")
            for p in range(4)
        ]
        # v (and the denominator ones-column) carry a 2^-12 scale so that
        # 1/denominator spans [6.6e-4, 4096] -- comfortably fp16 -- letting
        # the denominator broadcast be a plain full-rate fp16 matmul; the
        # 2^12 cancels exactly in y * (1/denom').
        VSC = 2.0 ** -12
        v_sb = persist.tile([128, NST, HPC, 65], f16, tag="v")
        nc.vector.memset(v_sb[:, :, :, 64:65], VSC)

        def kqv_mm(ps, w_sb, p, xt):
            for ci in range(8):
                nc.tensor.matmul(
                    ps,
                    lhsT=w_sb[:, ci, p * 128 : (p + 1) * 128],
                    rhs=xt[:, ci, :],
                    start=(ci == 0),
                    stop=(ci == 7),
                )

        def proj_rope(xt, p, tsl, ss_q, ss_k, ro_q, ro_k):
            """q,k for head pair p: projection, squares into shared stat
            tiles, RoPE combine into staging (normalization applied later)."""
            ps_q = psA.tile([128, TC], f32, tag="kqv")
            kqv_mm(ps_q, wq_sb, p, xt)
            qraw = work.tile([128, TC], f16, tag="qraw")
            nc.vector.tensor_copy(qraw, ps_q)
            sq_q = work.tile([128, TC], f16, tag="sq_q")
            nc.vector.tensor_mul(sq_q, qraw, qraw)
            # RoPE preserves row norms -> sums of squares from pre-RoPE values
            # (eps=1e-6 on rms~1 is far below fp16 noise; dropped).
            # M=32 with zero weight columns 2:32: rows 32p+2..32p+32 get
            # computed zeros, so all 128 stat rows are written (the batched
            # Ln below reads whole tiles; ln(0) rows are never used).
            nc.tensor.matmul(
                ss_q[32 * p : 32 * p + 32, :],
                lhsT=oc_sb,
                rhs=sq_q,
                start=True,
                stop=True,
                tile_position=(0, 32 * p),
            )

            ps_k = psA.tile([128, TC], f32, tag="kqv")
            kqv_mm(ps_k, wk_sb, p, xt)
            kraw = work.tile([128, TC], f16, tag="kraw")
            nc.vector.tensor_copy(kraw, ps_k)
            sq_k = work.tile([128, TC], f16, tag="sq_k")
            nc.vector.tensor_mul(sq_k, kraw, kraw)
            nc.tensor.matmul(
                ss_k[32 * p : 32 * p + 32, :],
                lhsT=oc_sb,
                rhs=sq_k,
                start=True,
                stop=True,
                tile_position=(0, 32 * p),
            )

            # rotate_half via signed permutation matmul on the PE;
            # ro = raw*cos + rot(raw)*sin staged un-normalized (the SBUF-only
            # cos-mul and add run on the otherwise idle Pool engine)
            for raw, dst in ((qraw, ro_q), (kraw, ro_k)):
                rot = psA.tile([128, TC], f32, tag="kqv")
                nc.tensor.matmul(rot, lhsT=p2_sb, rhs=raw, start=True, stop=True)
                qsh = work.tile([128, TC], f16, tag="qsh")
                nc.vector.tensor_mul(qsh, rot, sin_sb[:, tsl])
                t1 = work.tile([128, TC], f16, tag="t1")
                nc.vector.tensor_mul(t1, raw, cos_sb[:, tsl])
                nc.vector.tensor_add(dst, t1, qsh)

        def phase_a_units(tci):
            """Phase A for chunk tci as 8 units interleavable between the
            previous chunk's phase-B heads."""
            tsl = slice(tci * TC, (tci + 1) * TC)
            st8 = {}

            def u_start():
                st8["xt"] = xpool.tile([128, 8, TC], f16, tag="x", name="xt")
                nc.sync.dma_start(st8["xt"], xT[tci])
                st8["ss_q"] = psS.tile([128, TC], f32, tag="s", name="ss_q")
                st8["ss_k"] = psS.tile([128, TC], f32, tag="s", name="ss_k")
                st8["ro_q"] = [
                    work.tile([128, TC], f16, tag=f"roq{p}", name=f"roq{p}")
                    for p in range(4)
                ]
                st8["ro_k"] = [
                    work.tile([128, TC], f16, tag=f"rok{p}", name=f"rok{p}")
                    for p in range(4)
                ]
                proj_rope(
                    st8["xt"], 0, tsl, st8["ss_q"], st8["ss_k"],
                    st8["ro_q"][0], st8["ro_k"][0],
                )

            def u_pair(p):
                def f():
                    proj_rope(
                        st8["xt"], p, tsl, st8["ss_q"], st8["ss_k"],
                        st8["ro_q"][p], st8["ro_k"][p],
                    )
                return f

            def u_v():
                for st in range(4):
                    pv = psA.tile([128, TC], f32, tag="kqv")
                    for ci in range(8):
                        nc.tensor.matmul(
                            pv,
                            lhsT=st8["xt"][:, ci, st * 128 : (st + 1) * 128],
                            rhs=wv_sb[:, ci, :],
                            start=(ci == 0),
                            stop=(ci == 7),
                        )
                    nc.vector.tensor_scalar_mul(
                        v_sb[:, tci * 4 + st, :, 0:64],
                        pv.rearrange("p (h d) -> p h d", h=HPC),
                        VSC,
                    )

            def u_stats():
                # batched rsqrt of rms stats: rr = exp(-0.5 ln ss); the 8
                # from 1/rms = 8/sqrt(ss) is folded into obw. Rows between
                # the 32p..32p+2 stat rows are uninitialized PSUM; their
                # ln/exp results are garbage but never read.
                ln_q = work.tile([128, TC], f32, tag="lnt")
                nc.scalar.activation(ln_q, st8["ss_q"], AF.Ln)
                rr_q = work.tile([128, TC], f16, tag="rrq", bufs=2)
                nc.scalar.activation(rr_q, ln_q, AF.Exp, scale=-0.5)
                ln_k = work.tile([128, TC], f32, tag="lnt")
                nc.scalar.activation(ln_k, st8["ss_k"], AF.Ln)
                rr_k = work.tile([128, TC], f16, tag="rrk", bufs=2)
                nc.scalar.activation(rr_k, ln_k, AF.Exp, scale=-0.5)
                st8["rr_q"], st8["rr_k"] = rr_q, rr_k

            def u_norm(plo, phi):
                def f():
                    # qT/kT = ro * broadcast(8*w*rr)
                    for p in range(plo, phi):
                        bc_q = psS.tile([128, TC], f32, tag="s", name="bc_q")
                        nc.tensor.matmul(
                            bc_q,
                            lhsT=obwq_sb[32 * p : 32 * p + 2, :],
                            rhs=st8["rr_q"][32 * p : 32 * p + 2, :],
                            start=True,
                            stop=True,
                            tile_position=(32 * p, 0),
                        )
                        nc.vector.tensor_mul(qT[p][:, tsl], st8["ro_q"][p], bc_q)
                        bc_k = psS.tile([128, TC], f32, tag="s", name="bc_k")
                        nc.tensor.matmul(
                            bc_k,
                            lhsT=obwk_sb[32 * p : 32 * p + 2, :],
                            rhs=st8["rr_k"][32 * p : 32 * p + 2, :],
                            start=True,
                            stop=True,
                            tile_position=(32 * p, 0),
                        )
                        nc.vector.tensor_mul(kT[p][:, tsl], st8["ro_k"][p], bc_k)
                return f

            return [
                u_start, u_pair(1), u_pair(2), u_pair(3),
                u_v, u_stats, u_norm(0, 2), u_norm(2, 4),
            ]

        def pair_b(tci, p):
            """Phase B for both heads of pair p: the two heads' score
            matmuls contract disjoint 64-row halves of the PE array
            (tile_position (0,0) / (64,0) auto-derived from the operand
            bases), so issued back-to-back they run concurrently."""
            tsl = slice(tci * TC, (tci + 1) * TC)
            n_s = 4 * (tci + 1)
            ps_y0 = psY.tile([128, TC], f32, tag="y", name="ps_y0")
            ps_y1 = psY.tile([128, TC], f32, tag="y", name="ps_y1")
            for idx, si in enumerate(range(n_s)):
                delta = si * 128 - tci * TC
                d = max(delta, 0)
                csl = slice(d, TC)
                ssl = slice(si * 128, (si + 1) * 128)
                qsl = slice(tci * TC + d, (tci + 1) * TC)
                ps_s0 = psB.tile([128, TC], f32, tag="sc", name="ps_s0")
                nc.tensor.matmul(
                    ps_s0[:, csl], lhsT=kT[p][0:64, ssl], rhs=qT[p][0:64, qsl],
                    start=True, stop=True,
                )
                ps_s1 = psB.tile([128, TC], f32, tag="sc", name="ps_s1")
                nc.tensor.matmul(
                    ps_s1[:, csl], lhsT=kT[p][64:128, ssl],
                    rhs=qT[p][64:128, qsl], start=True, stop=True,
                )
                at0 = attp.tile([128, TC], f16, tag="at", name="at0")
                nc.scalar.activation(at0[:, csl], ps_s0[:, csl], AF.Exp, scale=0.125)
                at1 = attp.tile([128, TC], f16, tag="at", name="at1")
                nc.scalar.activation(at1[:, csl], ps_s1[:, csl], AF.Exp, scale=0.125)
                if delta >= 0:
                    for at in (at0, at1):
                        nc.gpsimd.tensor_mul(
                            at[:, d : d + 128], at[:, d : d + 128],
                            mask_sb[:, 0, 0:128],
                        )
                nc.tensor.matmul(
                    ps_y0[0:65, csl], lhsT=v_sb[:, si, 2 * p, 0:65],
                    rhs=at0[:, csl], start=(idx == 0), stop=(idx == n_s - 1),
                )
                nc.tensor.matmul(
                    ps_y1[0:65, csl], lhsT=v_sb[:, si, 2 * p + 1, 0:65],
                    rhs=at1[:, csl], start=(idx == 0), stop=(idx == n_s - 1),
                )
            for hl, ps_y in ((0, ps_y0), (1, ps_y1)):
                ystg = work.tile([65, TC], f32, tag="ystg")
                nc.vector.tensor_copy(ystg[0:64, :], ps_y[0:64, :])
                recw = work.tile([65, TC], f16, tag="recw")
                with nc.allow_low_precision(reason="1/denom' in [6.6e-4,4096]"):
                    nc.vector.reciprocal(recw[64:65, :], ps_y[64:65, :])
                nc.tensor.matmul(
                    ps_y[0:64, :], lhsT=ones16[64:65, :], rhs=recw[64:65, :],
                    start=True, stop=True,
                )
                if hl == 0:
                    nc.vector.tensor_mul(
                        yT[p][0:64, tsl], ystg[0:64, :], ps_y[0:64, :]
                    )
                else:
                    y16 = work.tile([64, TC], f16, tag="y16")
                    nc.vector.tensor_mul(y16, ystg[0:64, :], ps_y[0:64, :])
                    nc.sync.dma_start(yT[p][64:128, tsl], y16)

        def head_b(tci, h):
            """Phase B for one head of chunk tci: scores, exp, causal mask,
            AV accumulation, softmax division."""
            tsl = slice(tci * TC, (tci + 1) * TC)
            n_s = 4 * (tci + 1)
            p, hl = h // 2, h % 2
            hsl = slice(hl * 64, (hl + 1) * 64)
            ps_y = psY.tile([128, TC], f32, tag="y")
            # interior blocks first; diagonal blocks (with their exp->mask->AV
            # dependency tail) last so the tail overlaps the next head
            sis = list(range(4 * tci, n_s)) if tci == 0 else (
                list(range(0, 4 * tci)) + list(range(4 * tci, n_s))
            )
            for idx, si in enumerate(sis):
                # diagonal blocks: columns below the diagonal offset d are
                # fully masked -- compute only the [d, TC) range (the idx==0
                # block is always full width, so every PSUM byte is written)
                delta = si * 128 - tci * TC
                d = max(delta, 0)
                csl = slice(d, TC)
                ps_s = psB.tile([128, TC], f32, tag="sc", name="ps_s")
                nc.tensor.matmul(
                    ps_s[:, csl],
                    lhsT=kT[p][hsl, si * 128 : (si + 1) * 128],
                    rhs=qT[p][hsl, tci * TC + d : (tci + 1) * TC],
                    start=True,
                    stop=True,
                )
                at = attp.tile([128, TC], f16, tag="at")
                nc.scalar.activation(at[:, csl], ps_s[:, csl], AF.Exp, scale=0.125)
                if delta >= 0:
                    # only the leading 128 columns of the valid range cross
                    # the diagonal; the triangular [128,128] mask is the
                    # first block of the offset-0 mask. SBUF-only, so it can
                    # run on the otherwise idle Pool engine.
                    nc.gpsimd.tensor_mul(
                        at[:, d : d + 128],
                        at[:, d : d + 128],
                        mask_sb[:, 0, 0:128],
                    )
                nc.tensor.matmul(
                    ps_y[0:65, csl],
                    lhsT=v_sb[:, si, h, 0:65],
                    rhs=at[:, csl],
                    start=(idx == 0),
                    stop=(idx == n_s - 1),
                )
            # softmax denominator: stage y to SBUF (frees the bank region),
            # fp16 reciprocal straight off PSUM row 64 (the 2^-12 v-scale
            # keeps 1/denom' in fp16 range), broadcast down 64 partitions
            # at full PE rate into the just-staged (hence free) PSUM
            # region, one multiply to fp16 yT.
            ystg = work.tile([65, TC], f32, tag="ystg")
            nc.vector.tensor_copy(ystg[0:64, :], ps_y[0:64, :])
            recw = work.tile([65, TC], f16, tag="recw")
            with nc.allow_low_precision(reason="1/denom' in [6.6e-4,4096]"):
                nc.vector.reciprocal(recw[64:65, :], ps_y[64:65, :])
            nc.tensor.matmul(
                ps_y[0:64, :],
                lhsT=ones16[64:65, :],
                rhs=recw[64:65, :],
                start=True,
                stop=True,
            )
            if hl == 0:
                nc.vector.tensor_mul(
                    yT[p][0:64, tsl], ystg[0:64, :], ps_y[0:64, :]
                )
            else:
                y16 = work.tile([64, TC], f16, tag="y16")
                nc.vector.tensor_mul(y16, ystg[0:64, :], ps_y[0:64, :])
                nc.sync.dma_start(yT[p][64:128, tsl], y16)

        def body():
            for u in phase_a_units(0):
                u()
            for tci in range(NTC):
                nxt = phase_a_units(tci + 1) if tci + 1 < NTC else []
                for pp in range(4):
                    pair_b(tci, pp)
                    for u in nxt[2 * pp : 2 * pp + 2]:
                        u()

            # ---- phase C: output projection (partials over this core's
            # channels) ----
            for st in range(NST):
                for co in range(2):
                    po = psA.tile([128, TC], f32, tag="kqv")
                    for p in range(4):
                        nc.tensor.matmul(
                            po,
                            lhsT=yT[p][:, st * 128 : (st + 1) * 128],
                            rhs=wp_sb[:, p, co * 512 : (co + 1) * 512],
                            start=(p == 0),
                            stop=(p == 3),
                        )
                    ot = outp.tile([128, TC], f32, tag="o")
                    if co == 0:
                        nc.vector.tensor_copy(ot, po)
                    else:
                        # ACT is idle in phase C; Copy is in every act table
                        nc.scalar.copy(ot, po)
                    nc.sync.dma_start(
                        outd[st * 128 : (st + 1) * 128, co * 512 : (co + 1) * 512],
                        ot,
                    )

        if loop_n is None:
            body()
        else:
            with tc.For_i(0, loop_n, 1):
                body()

    return nc


def _get_nc(loop_n=None):
    key = ("nc", loop_n)
    if key not in _STATE:
        nc = _build_nc(loop_n)
        nc.finalize()
        _STATE[key] = nc
    return _STATE[key]


def _rope_tables():
    inv_freq = 1.0 / (10000.0 ** (np.arange(0, D, 2, dtype=np.float64) / D))
    t_pos = np.arange(T, dtype=np.float64)
    freqs = t_pos[:, None] * inv_freq[None, :]  # [T, 32]
    f2 = np.concatenate([freqs, freqs], axis=-1)  # [T, 64]
    cosT = np.cos(f2).T.astype(np.float16)  # [64, T]
    sinT = np.sin(f2).T.astype(np.float16)
    cos2 = np.concatenate([cosT, cosT], axis=0)  # [128, T]
    sin2 = np.concatenate([sinT, sinT], axis=0)
    return np.ascontiguousarray(cos2), np.ascontiguousarray(sin2)


def _prep_inputs(x, W_kqv, W_proj, q_norm_w, k_norm_w):
    x = np.asarray(x, dtype=np.float32)
    W_kqv = np.asarray(W_kqv, dtype=np.float32)
    W_proj = np.asarray(W_proj, dtype=np.float32)
    q_norm_w = np.asarray(q_norm_w, dtype=np.float32)
    k_norm_w = np.asarray(k_norm_w, dtype=np.float32)

    cos2, sin2 = _rope_tables()

    # causal masks for the 4 diagonal-crossing tile offsets
    si = np.arange(128)[:, None]
    tj = np.arange(TC)[None, :]
    mask = np.stack(
        [(tj >= si + 128 * o).astype(np.float16) for o in range(4)], axis=1
    )  # [128, 4, TC]

    # columns 0/1 sum the two heads' squares; columns 2:32 are zero weights
    # whose computed-zero outputs initialize the unused stat-tile rows
    oc = np.zeros((128, 32), dtype=np.float16)
    oc[0:64, 0] = 1.0
    oc[64:128, 1] = 1.0

    def obw8(w):
        # broadcast weights with the 8 = sqrt(D) of 1/rms folded in, one
        # 2-row block per head pair at partition 32p
        m = np.zeros((98, 128), dtype=np.float16)
        for p in range(4):
            m[32 * p + 0, 0:64] = 8.0 * w
            m[32 * p + 1, 64:128] = 8.0 * w
        return m

    # signed rotate-half permutation (per 64-dim head, stacked twice)
    P = np.zeros((64, 64), dtype=np.float16)
    for i in range(32):
        P[i, i + 32] = -1.0
        P[i + 32, i] = 1.0
    P2 = np.zeros((128, 128), dtype=np.float16)
    P2[0:64, 0:64] = P
    P2[64:128, 64:128] = P
    p2T = np.ascontiguousarray(P2.T)

    def wt_kqv(rows):
        # rows: [512, 1024] -> lhsT layout [128, 8, 512] fp16
        wT = rows.T.astype(np.float16)  # [1024, 512]
        return np.ascontiguousarray(wT.reshape(8, 128, 512).transpose(1, 0, 2))

    Wk, Wq, Wv = W_kqv[0:C], W_kqv[C : 2 * C], W_kqv[2 * C : 3 * C]

    in_maps = []
    for c in range(N_CORES):
        b, g = c // 2, c % 2
        rs = slice(512 * g, 512 * (g + 1))
        xTb = x[b].T.astype(np.float16)  # [C, T]
        xTr = np.ascontiguousarray(
            xTb.reshape(8, 128, NTC, TC).transpose(2, 1, 0, 3)
        )  # [NTC, 128, 8, TC]
        wp = W_proj[:, rs].T.astype(np.float16)  # [512, 1024]
        wpr = np.ascontiguousarray(wp.reshape(4, 128, 1024).transpose(1, 0, 2))
        in_maps.append(
            {
                "xT": xTr,
                "wqT": wt_kqv(Wq[rs]),
                "wkT": wt_kqv(Wk[rs]),
                "wvT": wt_kqv(Wv[rs]),
                "wpT": wpr,
                "cosd": cos2,
                "sind": sin2,
                "maskd": mask,
                "p2d": p2T,
                "ocd": oc,
                "obwqd": obw8(q_norm_w),
                "obwkd": obw8(k_norm_w),
            }
        )
    return in_maps


def _get_runner(loop_n=None):
    """Build (once) a cached jitted SPMD runner mirroring
    bass2jax.run_bass_via_pjrt, so repeated calls reuse the compiled NEFF."""
    key = ("runner", loop_n)
    if key in _STATE:
        return _STATE[key]

    import jax
    import concourse.mybir as mybir
    from concourse import bass2jax
    from concourse.bass2jax import _bass_exec_p, partition_id_tensor
    from jax.experimental.shard_map import shard_map
    from jax.sharding import Mesh, NamedSharding, PartitionSpec

    bass2jax.install_neuronx_cc_hook()
    nc = _get_nc(loop_n)

    partition_name = nc.partition_id_tensor.name if nc.partition_id_tensor else None
    in_names, out_names, out_avals, zero_outs = [], [], [], []
    for alloc in nc.m.functions[0].allocations:
        if not isinstance(alloc, mybir.MemoryLocationSet):
            continue
        name = alloc.memorylocations[0].name
        if alloc.kind == "ExternalInput":
            if name != partition_name:
                in_names.append(name)
        elif alloc.kind == "ExternalOutput":
            shape = tuple(alloc.tensor_shape)
            dtype = mybir.dt.np(alloc.dtype)
            out_names.append(name)
            out_avals.append(jax.core.ShapedArray(shape, dtype))
            zero_outs.append(np.zeros(shape, dtype))
    n_params = len(in_names)
    all_names = in_names + out_names
    if partition_name is not None:
        all_names.append(partition_name)

    def _body(*args):
        operands = list(args)
        if partition_name is not None:
            operands.append(partition_id_tensor())
        outs = _bass_exec_p.bind(
            *operands,
            out_avals=tuple(out_avals),
            in_names=tuple(all_names),
            out_names=tuple(out_names),
            lowering_input_output_aliases=(),
            sim_require_finite=True,
            sim_require_nnan=True,
            nc=nc,
        )
        return tuple(outs)

    devices = jax.devices()[:N_CORES]
    mesh = Mesh(np.asarray(devices), ("core",))
    spec = PartitionSpec("core")
    n_outs = len(out_names)
    sharded = jax.jit(
        shard_map(
            _body,
            mesh=mesh,
            in_specs=(spec,) * (n_params + n_outs),
            out_specs=(spec,) * n_outs,
            check_rep=False,
        ),
        keep_unused=True,
    )
    sharding = NamedSharding(mesh, spec)
    zeros_dev = [
        jax.device_put(
            np.zeros((N_CORES * z.shape[0], *z.shape[1:]), z.dtype), sharding
        )
        for z in zero_outs
    ]
    runner = {
        "sharded": sharded,
        "in_names": in_names,
        "out_names": out_names,
        "out_avals": out_avals,
        "zeros_dev": zeros_dev,
        "sharding": sharding,
    }
    _STATE[key] = runner
    return runner


def _concat_inputs(in_maps, runner):
    return [
        np.concatenate([np.asarray(in_maps[c][n]) for c in range(N_CORES)], axis=0)
        for n in runner["in_names"]
    ]


def _execute(in_maps):
    """Returns list (per core) of {out_name: np.ndarray}."""
    runner = _get_runner()
    concat_in = _concat_inputs(in_maps, runner)
    out_arrs = runner["sharded"](*concat_in, *runner["zeros_dev"])
    return [
        {
            n: np.asarray(out_arrs[i]).reshape(
                N_CORES, *runner["out_avals"][i].shape
            )[c]
            for i, n in enumerate(runner["out_names"])
        }
        for c in range(N_CORES)
    ]


def _wall(runner, in_maps, iters):
    import time
    import jax

    concat_in = [
        jax.device_put(a, runner["sharding"])
        for a in _concat_inputs(in_maps, runner)
    ]
    args = (*concat_in, *runner["zeros_dev"])
    jax.block_until_ready(runner["sharded"](*args))  # warmup
    times = []
    for _ in range(iters):
        t0 = time.perf_counter()
        jax.block_until_ready(runner["sharded"](*args))
        times.append(time.perf_counter() - t0)
    times.sort()
    return times


def _timed(in_maps, iters=20, n_lo=1, n_hi=33):
    """Per-pass HW time via two device-side repeat counts: the dispatch/tunnel
    overhead cancels in the difference."""
    r_lo = _get_runner(None if n_lo == 1 else n_lo)
    r_hi = _get_runner(n_hi)
    t_lo = _wall(r_lo, in_maps, iters)
    t_hi = _wall(r_hi, in_maps, iters)
    k = max(3, iters // 4)
    lo = sum(t_lo[:k]) / k
    hi = sum(t_hi[:k]) / k
    per_pass = (hi - lo) / (n_hi - n_lo)
    return per_pass, lo, hi


def kernel(**inputs):
    in_maps = _prep_inputs(**inputs)
    res = _execute(in_maps)
    out = np.zeros((B, T, C), dtype=np.float32)
    for c in range(N_CORES):
        out[c // 2] += res[c]["out"]
    return out
